# revision 30
# baseline (speedup 1.0000x reference)
"""nn_ContrastiveLoss Trainium2 kernel (8 NeuronCores, data-parallel over batch).

Contract: kernel(embeddings=[64,1024,128] f32, labels=[64,1024] int64) -> f32 scalar.

v6 design. Host does all O(N*D) work: L2-normalize, split rows by label
(smaller side is the matmul stationary operand, always <= 512), transpose
to [D, rows] layout, cast bf16. Upload per sample is one [128, 512+m_pad]
bf16 tile: D=128 on partitions, stationary cols [0:512), moving cols
[512:512+m_pad), zero-padded.

Device runs only the O(N^2*D) part, exploiting PE column-group packing:
  - mains: per sample, subtiles 2j / 2j+1 (64 stationary cols each) are two
    M=64 matmuls into col groups 0 / 64 of ONE PSUM bank; the PE streams
    their 512 moving cols CONCURRENTLY (one 512-cycle pass per bank).
    4 banks per sample, every matmul bank-aligned N=512.
  - tails: moving cols beyond 512 (<= 32) flip roles: lhsT = tail cols,
    rhs = the 512 stationary cols; a QUAD of samples packs into one bank at
    32-partition offsets (two concurrent col-group pairs).
  - hinge fused with reduce: per sample one ACT op (relu(sim-t) with
    accum_out) and one DVE op (max(sim,t)-accumulate, offset removed
    host-side); tail banks get one op per quad, attributed per partition.
Host: per-sample division by max(nneg,1), validity, final count division.
"""

import sys

if "/opt/trn_rl_repo" not in sys.path:
    sys.path.insert(0, "/opt/trn_rl_repo")

from contextlib import ExitStack

import numpy as np

import concourse.bass as bass
import concourse.bacc as bacc
import concourse.mybir as mybir
import concourse.tile as tile
from concourse import bass_utils

F32 = mybir.dt.float32
BF16 = mybir.dt.bfloat16
AF = mybir.ActivationFunctionType
ALU = mybir.AluOpType

P = 128      # SBUF partitions
D = 128      # embedding dim
N = 1024     # rows per sample
B = 64       # full batch
NCORES = 8
BPC = B // NCORES
THRESH = 0.5 - 0.35   # margin threshold 0.15
SPAD = 512            # stationary side pad (min side always <= 512)
RING = 4096           # PSUM ring, f32 cols (8 banks x 512)
ACT_SHARE = 1024      # per-sample hinge elems on ACT (rest on DVE)
TAILPART = (0, 64, 32, 96)  # partition base of sample i-of-quad in tail bank
NWARM = 3


def _plan(m_pad):
    """Layout plan: per-sample main banks, per-quad tail bank, hinge ops.

    hinge ops: (eng, kind, idx, pieces, slot)
      eng 0=ACT 1=DVE; kind 'main' (idx=sample) | 'tail' (idx=quad);
      pieces = [(f32_off, width)] in the flat PSUM ring.
    """
    r_pad = m_pad - SPAD
    assert 0 <= r_pad <= 32, "tail packing assumes remainder <= 32 cols"
    cur = 0
    main_banks, tail_banks, hinge = [], [], []
    slot = 0

    def pieces_of(base_f32, lo, hi):
        out = []
        done = lo
        while done < hi:
            oo = (base_f32 + done) % RING
            w = min(RING - oo, hi - done)
            out.append((oo, w))
            done += w
        return out

    for b in range(BPC):
        banks = [(cur + j) % 8 for j in range(4)]
        main_banks.append(banks)
        base = (cur % 8) * 512
        hinge.append((0, "main", b, pieces_of(base, 0, ACT_SHARE), slot))
        slot += len(hinge[-1][3])
        hinge.append((1, "main", b, pieces_of(base, ACT_SHARE, 2048), slot))
        slot += len(hinge[-1][3])
        cur += 4
        if r_pad and b % 4 == 3:
            q = b // 4
            tb = cur % 8
            tail_banks.append(tb)
            # split the tail bank across both engines so the final quad's
            # hinge drains in parallel
            hinge.append((0, "tail", q, [(tb * 512, 256)], slot))
            slot += 1
            hinge.append((1, "tail", q, [(tb * 512 + 256, 256)], slot))
            slot += 1
            cur += 1
    return {"main_banks": main_banks, "tail_banks": tail_banks,
            "hinge": hinge, "n_slots": slot, "r_pad": r_pad}


def _kernel_body(ctx, tc, emb_ap, out_ap, m_pad, plan):
    nc = tc.nc
    W = SPAD + m_pad
    r_pad = plan["r_pad"]

    const_pool = ctx.enter_context(tc.tile_pool(name="const", bufs=1))
    epool = ctx.enter_context(tc.tile_pool(name="epool", bufs=BPC))
    acc_pool = ctx.enter_context(tc.tile_pool(name="acc", bufs=1))
    ring_pool = ctx.enter_context(tc.tile_pool(name="ring", bufs=1,
                                               space="PSUM"))

    warm = const_pool.tile([P, 512], BF16)
    nc.vector.memset(warm[:], 0.125)
    neg_thr = const_pool.tile([P, 1], F32)
    nc.vector.memset(neg_thr[:], -THRESH)
    warmf = const_pool.tile([P, 1], F32)
    nc.vector.memset(warmf[:], 1.0)
    # pull the Relu ACT table load forward into the DMA wait
    nc.scalar.activation(warmf[:], warmf[:], AF.Relu, bias=neg_thr[:])

    slots = acc_pool.tile([P, plan["n_slots"]], F32)
    nc.gpsimd.memset(slots[:], 0.0)

    ring = ring_pool.tile([P, RING // 512, 512], F32)
    rf = ring[:].rearrange("p a w -> p (a w)")

    # Input DMAs on SP in consumption order. The first three are chained
    # via tiny GPSIMD gate-copies (WAW on the next tile) so et0 gets the
    # full DMA bandwidth instead of sharing it 8 ways; et3..7 then flow
    # freely (they are consumed much later).
    ets = {}
    for b in range(BPC):
        ets[b] = epool.tile([P, W], BF16, tag="et", name=f"et{b}")
    for b in range(BPC):
        if b in (1, 2):
            nc.gpsimd.tensor_copy(ets[b][:, 0:1], ets[b - 1][:, 0:1])
        nc.sync.dma_start(ets[b][:], emb_ap[b, :, :])

    # PE warmup during the first DMA (HAM spin-up); writes land in ring
    # bank 7, overwritten by sample 1's matmuls before any hinge reads.
    for _ in range(NWARM):
        nc.tensor.matmul(ring[0:P, 7, :], lhsT=warm[:, 0:128],
                         rhs=warm[:], start=True, stop=True)

    hinge_by = {}
    for op in plan["hinge"]:
        hinge_by.setdefault((op[1], op[2]), []).append(op)

    def emit_hinge(op):
        eng, kind, idx, pcs, slot = op
        for k, (oo, w) in enumerate(pcs):
            view = rf[:, oo:oo + w]
            sl = slots[:, slot + k:slot + k + 1]
            if eng == 0:
                nc.scalar.activation(view, view, AF.Relu, bias=neg_thr[:],
                                     accum_out=sl)
            else:
                nc.vector.tensor_scalar(view, view, THRESH, None,
                                        ALU.max, ALU.add, accum_out=sl)

    for b in range(BPC):
        et = ets[b]
        for j, bank in enumerate(plan["main_banks"][b]):
            nc.tensor.matmul(ring[0:64, bank, :],
                             lhsT=et[:, 128 * j:128 * j + 64],
                             rhs=et[:, SPAD:SPAD + 512],
                             start=True, stop=True)
            nc.tensor.matmul(ring[64:128, bank, :],
                             lhsT=et[:, 128 * j + 64:128 * j + 128],
                             rhs=et[:, SPAD:SPAD + 512],
                             start=True, stop=True)
        for op in hinge_by.get(("main", b), []):
            emit_hinge(op)
        if r_pad and b % 4 == 3:
            q = b // 4
            tb = plan["tail_banks"][q]
            for i in range(4):
                s = 4 * q + i
                pp = TAILPART[i]
                nc.tensor.matmul(
                    ring[pp:pp + r_pad, tb, :],
                    lhsT=ets[s][:, SPAD + 512:SPAD + 512 + r_pad],
                    rhs=ets[s][:, 0:SPAD],
                    start=True, stop=True,
                    tile_position=(0, pp))
            for op in hinge_by.get(("tail", q), []):
                emit_hinge(op)
        if b == BPC - 3:
            # stream out the slots finished so far; shrinks the tail DMA
            ksp = min(op[4] for op in plan["hinge"]
                      if (op[1] == "main" and op[2] >= BPC - 2)
                      or (op[1] == "tail" and op[2] == BPC // 4 - 1))
            nc.sync.dma_start(out_ap[:, 0:ksp], slots[:, 0:ksp])
    nc.sync.dma_start(out_ap[:, ksp:], slots[:, ksp:])


def _ap_key(arg):
    try:
        return (getattr(arg, "memref", None), getattr(arg, "offset", None),
                str(getattr(arg, "ap", None)), str(getattr(arg, "dtype", None)))
    except Exception:
        return None


def _dedup_ldweights(nc):
    """Remove InstLdweights that reload weights already in the array."""
    removed = 0
    for fn in nc.m.functions:
        for bb in fn.blocks:
            prev_key = None
            victims = []
            for x in list(bb.instructions):
                tn = type(x).__name__
                eng = getattr(x, "engine", None)
                if tn == "InstLdweights":
                    key = _ap_key(x.ins[0])
                    try:
                        clean = (not x.has_wait()) and (not x.has_update())
                    except Exception:
                        clean = False
                    if key is not None and key == prev_key and clean \
                            and not getattr(x, "is_transpose", False):
                        victims.append(x)
                    else:
                        prev_key = key
                elif tn == "InstMatmult":
                    if getattr(x, "is_transpose", None):
                        prev_key = None
                elif eng is not None and "PE" in str(eng):
                    prev_key = None
            for v in victims:
                bb.instructions.remove(v)
                removed += 1
    return removed


_NC_CACHE = {}


def _build(m_pad):
    key = (BPC, NCORES, m_pad, ACT_SHARE, NWARM)
    if key in _NC_CACHE:
        return _NC_CACHE[key]
    W = SPAD + m_pad
    plan = _plan(m_pad)
    nc = bacc.Bacc("TRN2", target_bir_lowering=False, debug=False,
                   num_devices=NCORES)
    emb = nc.dram_tensor("emb", [BPC, P, W], BF16, kind="ExternalInput")
    out = nc.dram_tensor("out", [P, plan["n_slots"]], F32,
                         kind="ExternalOutput")
    with tile.TileContext(nc) as tc:
        with ExitStack() as ctx:
            _kernel_body(ctx, tc, emb.ap(), out.ap(), m_pad, plan)
    nc.compile()
    _dedup_ldweights(nc)
    _NC_CACHE[key] = nc
    return nc


def _pack(emb, labels):
    """Normalize, label-split (min side stationary), transpose, cast bf16."""
    import ml_dtypes

    npos = (labels == 1).sum(axis=1).astype(np.int64)
    nneg = N - npos
    mmax = int(np.maximum(npos, nneg).max())
    m_pad = -(-mmax // 32) * 32
    W = SPAD + m_pad

    nrm = np.sqrt((emb.astype(np.float64) ** 2).sum(axis=2))
    e_n = (emb / np.maximum(nrm, 1e-12)[:, :, None]).astype(np.float32)

    eT = np.zeros((B, P, W), dtype=ml_dtypes.bfloat16)
    for b in range(B):
        pos_idx = np.nonzero(labels[b] == 1)[0]
        neg_idx = np.nonzero(labels[b] == 0)[0]
        if len(pos_idx) <= len(neg_idx):
            stat, mov = pos_idx, neg_idx
        else:
            stat, mov = neg_idx, pos_idx
        eT[b, :, :len(stat)] = e_n[b, stat].T
        eT[b, :, SPAD:SPAD + len(mov)] = e_n[b, mov].T
    return eT, m_pad, npos, nneg


def kernel(embeddings: np.ndarray, labels: np.ndarray,
           _want_results=False, _trace=False) -> np.ndarray:
    emb = np.ascontiguousarray(embeddings, dtype=np.float32)
    lab = np.asarray(labels)
    assert emb.shape == (B, N, D) and lab.shape == (B, N)

    eT, m_pad, npos, nneg = _pack(emb, lab)
    plan = _plan(m_pad)
    nc = _build(m_pad)
    in_maps = [{"emb": eT[c * BPC:(c + 1) * BPC]} for c in range(NCORES)]
    res = bass_utils.run_bass_kernel_spmd(nc, in_maps,
                                          core_ids=list(range(NCORES)),
                                          trace=_trace)

    inv_nneg = np.array([1.0 / max(float(v), 1.0) for v in nneg])
    valid = (npos > 0) & (nneg > 0)
    loss_sum = 0.0
    for ci in range(NCORES):
        slots = np.asarray(res.results[ci]["out"], np.float64)  # [P, ns]
        for (eng, kind, idx, pcs, slot) in plan["hinge"]:
            fd = sum(w for (_, w) in pcs)
            nsl = len(pcs)
            if kind == "main":
                b = ci * BPC + idx
                if not valid[b]:
                    continue
                s = slots[:, slot:slot + nsl].sum()
                if eng == 1:
                    s -= THRESH * fd * P
                loss_sum += s * inv_nneg[b]
            else:
                col = slots[:, slot]
                if eng == 1:
                    col = col - THRESH * fd
                # partition p belongs to sample 4q + TAILPART-slot owner
                for i in range(4):
                    b = ci * BPC + 4 * idx + i
                    if not valid[b]:
                        continue
                    pp = TAILPART[i]
                    loss_sum += col[pp:pp + plan["r_pad"]].sum() * inv_nneg[b]
    count = float((npos * valid).sum())
    ans = np.float32(loss_sum / max(count, 1.0))
    if _want_results:
        return ans, res
    return ans


# revision 31
# speedup vs baseline: 1.1869x; 1.1869x over previous
"""nn_ContrastiveLoss Trainium2 kernel (8 NeuronCores, data-parallel over batch).

Contract: kernel(embeddings=[64,1024,128] f32, labels=[64,1024] int64) -> f32 scalar.

v6 design. Host does all O(N*D) work: L2-normalize, split rows by label
(smaller side is the matmul stationary operand, always <= 512), transpose
to [D, rows] layout, cast bf16. Upload per sample is one [128, 512+m_pad]
bf16 tile: D=128 on partitions, stationary cols [0:512), moving cols
[512:512+m_pad), zero-padded.

Device runs only the O(N^2*D) part, exploiting PE column-group packing:
  - mains: per sample, subtiles 2j / 2j+1 (64 stationary cols each) are two
    M=64 matmuls into col groups 0 / 64 of ONE PSUM bank; the PE streams
    their 512 moving cols CONCURRENTLY (one 512-cycle pass per bank).
    4 banks per sample, every matmul bank-aligned N=512.
  - tails: moving cols beyond 512 (<= 32) flip roles: lhsT = tail cols,
    rhs = the 512 stationary cols; a QUAD of samples packs into one bank at
    32-partition offsets (two concurrent col-group pairs).
  - hinge fused with reduce: per sample one ACT op (relu(sim-t) with
    accum_out) and one DVE op (max(sim,t)-accumulate, offset removed
    host-side); tail banks get one op per quad, attributed per partition.
Host: per-sample division by max(nneg,1), validity, final count division.
"""

import sys

if "/opt/trn_rl_repo" not in sys.path:
    sys.path.insert(0, "/opt/trn_rl_repo")

from contextlib import ExitStack

import numpy as np

import concourse.bass as bass
import concourse.bacc as bacc
import concourse.mybir as mybir
import concourse.tile as tile
from concourse import bass_utils

F32 = mybir.dt.float32
BF16 = mybir.dt.bfloat16
AF = mybir.ActivationFunctionType
ALU = mybir.AluOpType

P = 128      # SBUF partitions
D = 128      # embedding dim
N = 1024     # rows per sample
B = 64       # full batch
NCORES = 8
BPC = B // NCORES
THRESH = 0.5 - 0.35   # margin threshold 0.15
SPAD = 512            # stationary side pad (min side always <= 512)
RING = 4096           # PSUM ring, f32 cols (8 banks x 512)
ACT_SHARE = 1024      # per-sample hinge elems on ACT (rest on DVE)
TAILPART = (0, 64, 32, 96)  # partition base of sample i-of-quad in tail bank
NWARM = 6


def _plan(m_pad):
    """Layout plan: per-sample main banks, per-quad tail bank, hinge ops.

    hinge ops: (eng, kind, idx, pieces, slot)
      eng 0=ACT 1=DVE; kind 'main' (idx=sample) | 'tail' (idx=quad);
      pieces = [(f32_off, width)] in the flat PSUM ring.
    """
    r_pad = m_pad - SPAD
    assert 0 <= r_pad <= 32, "tail packing assumes remainder <= 32 cols"
    cur = 0
    main_banks, tail_banks, hinge = [], [], []
    slot = 0

    def pieces_of(base_f32, lo, hi):
        out = []
        done = lo
        while done < hi:
            oo = (base_f32 + done) % RING
            w = min(RING - oo, hi - done)
            out.append((oo, w))
            done += w
        return out

    for b in range(BPC):
        banks = [(cur + j) % 8 for j in range(4)]
        main_banks.append(banks)
        base = (cur % 8) * 512
        hinge.append((0, "main", b, pieces_of(base, 0, ACT_SHARE), slot))
        slot += len(hinge[-1][3])
        hinge.append((1, "main", b, pieces_of(base, ACT_SHARE, 2048), slot))
        slot += len(hinge[-1][3])
        cur += 4
        if r_pad and b % 4 == 3:
            q = b // 4
            tb = cur % 8
            tail_banks.append(tb)
            # split the tail bank across both engines so the final quad's
            # hinge drains in parallel
            hinge.append((0, "tail", q, [(tb * 512, 256)], slot))
            slot += 1
            hinge.append((1, "tail", q, [(tb * 512 + 256, 256)], slot))
            slot += 1
            cur += 1
    return {"main_banks": main_banks, "tail_banks": tail_banks,
            "hinge": hinge, "n_slots": slot, "r_pad": r_pad}


def _kernel_body(ctx, tc, emb_ap, out_ap, m_pad, plan):
    nc = tc.nc
    W = SPAD + m_pad
    r_pad = plan["r_pad"]

    const_pool = ctx.enter_context(tc.tile_pool(name="const", bufs=1))
    epool = ctx.enter_context(tc.tile_pool(name="epool", bufs=BPC))
    acc_pool = ctx.enter_context(tc.tile_pool(name="acc", bufs=1))
    ring_pool = ctx.enter_context(tc.tile_pool(name="ring", bufs=1,
                                               space="PSUM"))

    warm = const_pool.tile([P, 512], BF16)
    nc.vector.memset(warm[:], 0.125)
    neg_thr = const_pool.tile([P, 1], F32)
    nc.vector.memset(neg_thr[:], -THRESH)
    warmf = const_pool.tile([P, 1], F32)
    nc.vector.memset(warmf[:], 1.0)
    # pull the Relu ACT table load forward into the DMA wait
    nc.scalar.activation(warmf[:], warmf[:], AF.Relu, bias=neg_thr[:])

    slots = acc_pool.tile([P, plan["n_slots"]], F32)
    nc.gpsimd.memset(slots[:], 0.0)

    ring = ring_pool.tile([P, RING // 512, 512], F32)
    rf = ring[:].rearrange("p a w -> p (a w)")

    # all input DMAs upfront on SP in consumption order
    ets = {}
    for b in range(BPC):
        ets[b] = epool.tile([P, W], BF16, tag="et", name=f"et{b}")
        nc.sync.dma_start(ets[b][:], emb_ap[b, :, :])

    # PE warmup during the first DMA (HAM spin-up); writes land in ring
    # bank 7, overwritten by sample 1's matmuls before any hinge reads.
    for _ in range(NWARM):
        nc.tensor.matmul(ring[0:P, 7, :], lhsT=warm[:, 0:128],
                         rhs=warm[:], start=True, stop=True)

    hinge_by = {}
    for op in plan["hinge"]:
        hinge_by.setdefault((op[1], op[2]), []).append(op)

    def emit_hinge(op):
        eng, kind, idx, pcs, slot = op
        for k, (oo, w) in enumerate(pcs):
            view = rf[:, oo:oo + w]
            sl = slots[:, slot + k:slot + k + 1]
            if eng == 0:
                nc.scalar.activation(view, view, AF.Relu, bias=neg_thr[:],
                                     accum_out=sl)
            else:
                nc.vector.tensor_scalar(view, view, THRESH, None,
                                        ALU.max, ALU.add, accum_out=sl)

    for b in range(BPC):
        et = ets[b]
        for j, bank in enumerate(plan["main_banks"][b]):
            nc.tensor.matmul(ring[0:64, bank, :],
                             lhsT=et[:, 128 * j:128 * j + 64],
                             rhs=et[:, SPAD:SPAD + 512],
                             start=True, stop=True)
            nc.tensor.matmul(ring[64:128, bank, :],
                             lhsT=et[:, 128 * j + 64:128 * j + 128],
                             rhs=et[:, SPAD:SPAD + 512],
                             start=True, stop=True)
        for op in hinge_by.get(("main", b), []):
            emit_hinge(op)
        if r_pad and b % 4 == 3:
            q = b // 4
            tb = plan["tail_banks"][q]
            for i in range(4):
                s = 4 * q + i
                pp = TAILPART[i]
                nc.tensor.matmul(
                    ring[pp:pp + r_pad, tb, :],
                    lhsT=ets[s][:, SPAD + 512:SPAD + 512 + r_pad],
                    rhs=ets[s][:, 0:SPAD],
                    start=True, stop=True,
                    tile_position=(0, pp))
            for op in hinge_by.get(("tail", q), []):
                emit_hinge(op)
        if b == BPC - 3:
            # stream out the slots finished so far; shrinks the tail DMA
            ksp = min(op[4] for op in plan["hinge"]
                      if (op[1] == "main" and op[2] >= BPC - 2)
                      or (op[1] == "tail" and op[2] == BPC // 4 - 1))
            nc.sync.dma_start(out_ap[:, 0:ksp], slots[:, 0:ksp])
    nc.sync.dma_start(out_ap[:, ksp:], slots[:, ksp:])


def _ap_key(arg):
    try:
        return (getattr(arg, "memref", None), getattr(arg, "offset", None),
                str(getattr(arg, "ap", None)), str(getattr(arg, "dtype", None)))
    except Exception:
        return None


def _dedup_ldweights(nc):
    """Remove InstLdweights that reload weights already in the array."""
    removed = 0
    for fn in nc.m.functions:
        for bb in fn.blocks:
            prev_key = None
            victims = []
            for x in list(bb.instructions):
                tn = type(x).__name__
                eng = getattr(x, "engine", None)
                if tn == "InstLdweights":
                    key = _ap_key(x.ins[0])
                    try:
                        clean = (not x.has_wait()) and (not x.has_update())
                    except Exception:
                        clean = False
                    if key is not None and key == prev_key and clean \
                            and not getattr(x, "is_transpose", False):
                        victims.append(x)
                    else:
                        prev_key = key
                elif tn == "InstMatmult":
                    if getattr(x, "is_transpose", None):
                        prev_key = None
                elif eng is not None and "PE" in str(eng):
                    prev_key = None
            for v in victims:
                bb.instructions.remove(v)
                removed += 1
    return removed


_NC_CACHE = {}


def _build(m_pad):
    key = (BPC, NCORES, m_pad, ACT_SHARE, NWARM)
    if key in _NC_CACHE:
        return _NC_CACHE[key]
    W = SPAD + m_pad
    plan = _plan(m_pad)
    nc = bacc.Bacc("TRN2", target_bir_lowering=False, debug=False,
                   num_devices=NCORES)
    emb = nc.dram_tensor("emb", [BPC, P, W], BF16, kind="ExternalInput")
    out = nc.dram_tensor("out", [P, plan["n_slots"]], F32,
                         kind="ExternalOutput")
    with tile.TileContext(nc) as tc:
        with ExitStack() as ctx:
            _kernel_body(ctx, tc, emb.ap(), out.ap(), m_pad, plan)
    nc.compile()
    _dedup_ldweights(nc)
    _NC_CACHE[key] = nc
    return nc


def _pack(emb, labels):
    """Normalize, label-split (min side stationary), transpose, cast bf16."""
    import ml_dtypes

    npos = (labels == 1).sum(axis=1).astype(np.int64)
    nneg = N - npos
    mmax = int(np.maximum(npos, nneg).max())
    m_pad = -(-mmax // 32) * 32
    W = SPAD + m_pad

    nrm = np.sqrt((emb.astype(np.float64) ** 2).sum(axis=2))
    e_n = (emb / np.maximum(nrm, 1e-12)[:, :, None]).astype(np.float32)

    eT = np.zeros((B, P, W), dtype=ml_dtypes.bfloat16)
    for b in range(B):
        pos_idx = np.nonzero(labels[b] == 1)[0]
        neg_idx = np.nonzero(labels[b] == 0)[0]
        if len(pos_idx) <= len(neg_idx):
            stat, mov = pos_idx, neg_idx
        else:
            stat, mov = neg_idx, pos_idx
        eT[b, :, :len(stat)] = e_n[b, stat].T
        eT[b, :, SPAD:SPAD + len(mov)] = e_n[b, mov].T
    return eT, m_pad, npos, nneg


def kernel(embeddings: np.ndarray, labels: np.ndarray,
           _want_results=False, _trace=False) -> np.ndarray:
    emb = np.ascontiguousarray(embeddings, dtype=np.float32)
    lab = np.asarray(labels)
    assert emb.shape == (B, N, D) and lab.shape == (B, N)

    eT, m_pad, npos, nneg = _pack(emb, lab)
    plan = _plan(m_pad)
    nc = _build(m_pad)
    in_maps = [{"emb": eT[c * BPC:(c + 1) * BPC]} for c in range(NCORES)]
    res = bass_utils.run_bass_kernel_spmd(nc, in_maps,
                                          core_ids=list(range(NCORES)),
                                          trace=_trace)

    inv_nneg = np.array([1.0 / max(float(v), 1.0) for v in nneg])
    valid = (npos > 0) & (nneg > 0)
    loss_sum = 0.0
    for ci in range(NCORES):
        slots = np.asarray(res.results[ci]["out"], np.float64)  # [P, ns]
        for (eng, kind, idx, pcs, slot) in plan["hinge"]:
            fd = sum(w for (_, w) in pcs)
            nsl = len(pcs)
            if kind == "main":
                b = ci * BPC + idx
                if not valid[b]:
                    continue
                s = slots[:, slot:slot + nsl].sum()
                if eng == 1:
                    s -= THRESH * fd * P
                loss_sum += s * inv_nneg[b]
            else:
                col = slots[:, slot]
                if eng == 1:
                    col = col - THRESH * fd
                # partition p belongs to sample 4q + TAILPART-slot owner
                for i in range(4):
                    b = ci * BPC + 4 * idx + i
                    if not valid[b]:
                        continue
                    pp = TAILPART[i]
                    loss_sum += col[pp:pp + plan["r_pad"]].sum() * inv_nneg[b]
    count = float((npos * valid).sum())
    ans = np.float32(loss_sum / max(count, 1.0))
    if _want_results:
        return ans, res
    return ans
